# revision 28
# baseline (speedup 1.0000x reference)
"""CBOW negative-sampling loss kernel for Trainium2 (8 NeuronCores).

Problem: nn_CBOWModel_18356690223611
    pos_u  [16384, 10] int  -- context word ids into u_weight
    pos_w  [16384]     int  -- target word ids into w_weight
    neg_w  [16384, 5]  int  -- negative sample ids into w_weight
    u_weight [100000, 128] f32
    w_weight [100000, 128] f32
    out = sum_b softplus(-dot(su_b, wpos_b)) + softplus(dot(su_b, wneg_sum_b))
      where su_b = sum_c u_weight[pos_u[b,c]], wneg_sum_b = sum_k w_weight[neg_w[b,k]]
    (equivalent to -(sum logsigmoid(pos) + sum logsigmoid(-neg)))

Sharding: data-parallel over batch, 2048 samples per core; embedding tables
replicated (concatenated into one [200000, 128] DRAM tensor) per core.

The gather is 256 SWDGE indirect DMAs per core. This is descgen-bound: the
indirect1d ISA consumes exactly one index per dest partition (its 1D tensor
descriptor cannot express multi-block-per-partition dests: the partition dim
is silently dropped, and single-partition multi-block forms cap at ~128
blocks with an N-dependent index-consumption layout, crashing the device at
256). This image's firmware runs all SWDGE descgen on one Q7 pair (~1.10us
Q7 time + ~0.31us dispatch per instruction, strictly serial; multi-queue
round-robin re-measured at zero gain -- every indirect op does an 8-Q7
barrier + index allgather; HWDGE RTL excludes indirection; the extended-ISA
dma_gather/ap_gather ucode is absent from this image and executes as
garbage). So the kernel keeps the descgen stream dense and hides everything
else under it:
  - all 256 gathers land in one 128KB/partition SBUF tile (no buffer reuse
    -> no WAR stalls),
  - dynamic_dma_scratch_size=48KB/partition (3x default) so the descriptor
    ring barely wraps -> no await_space reclaim stalls mid-stream (8-9
    straggler gathers at 16KB, ~4 at 32KB, ~1 at 48KB; 56KB + all-W=1
    chunks measured WORSE overall, +6.6us),
  - the index tile is preloaded on the sync queue's HWDGE (chunk-0 columns
    in a first small DMA so descgen starts as early as possible),
  - DVE tree-sums/dots, ACT softplus and the running row-accumulate trail
    the gather stream per chunk (softplus is per-chunk so it is hidden);
    chunks are 2 sample-columns wide (keeps work tiles small enough for the
    larger ring carveout) with two final 1-wide chunks, and the last
    chunk's emission is interleaved with its gathers (u rows -> context
    tree -> neg rows -> neg tree+softplus -> pos row last) so only the
    pos-score chain + PE cross-partition sum + out-DMA remain after the
    final gather.
"""

import numpy as np

VOCAB = 100000
DIM = 128
B = 16384
CTX = 10
NEG = 5
WK = NEG + 1  # pos + neg lookups into w_weight per sample
NIDX = CTX + WK  # 16 gathered rows per sample

N_CORES = 8
BPC = B // N_CORES  # 2048 samples per core
P = 128
TILES = BPC // P  # 16 sample columns of 128 samples
CHUNK_WIDTHS = (2, 2, 2, 2, 2, 2, 2, 1, 1)  # sample columns per pipeline chunk
NS = 2 * TILES  # score columns: (pos, neg) per sample column

_CACHE = {}


def _build_nc():
    import concourse.bacc as bacc
    import concourse.bass as bass
    import concourse.mybir as mybir
    import concourse.tile as tile

    f32 = mybir.dt.float32
    i32 = mybir.dt.int32
    ADD = mybir.AluOpType.add
    MUL = mybir.AluOpType.mult

    # 48KB/partition DMA-descriptor-ring carveout (default 16KB): the Q7
    # descgen takes a ~1.5-2.5us reclaim stall each time the ring wraps
    # (every ~2 instructions per KB), visible as straggler DMA_INDIRECTs in
    # the trace (8-9 stragglers at 16KB, ~4 at 32KB). Narrow chunks (W=2)
    # shrink the DVE work tiles to make the SBUF room for this.
    nc = bacc.Bacc("TRN2", target_bir_lowering=False, debug=False,
                   enable_asserts=False, dynamic_dma_scratch_size=53248)

    idx_d = nc.dram_tensor("idx", [P, NIDX * TILES], i32,
                           kind="ExternalInput")
    uw_w = nc.dram_tensor("uw_weight", [2 * VOCAB, DIM], f32,
                          kind="ExternalInput")
    out_d = nc.dram_tensor("out", [1, 1], f32, kind="ExternalOutput")

    with tile.TileContext(nc) as tc:
        with (
            tc.tile_pool(name="idx", bufs=1) as idxp,
            tc.tile_pool(name="g", bufs=1) as gpool,
            tc.tile_pool(name="work", bufs=2) as work,
            tc.tile_pool(name="accum", bufs=1) as accp,
            tc.tile_pool(name="psum", bufs=1, space="PSUM") as psp,
        ):
            idx_t = idxp.tile([P, NIDX * TILES], i32)
            # idx load on the sync queue's HWDGE; chunk-0 columns in a first
            # small DMA so gather descgen can start as soon as they land
            # (measured: the scalar/Act queue is ~6us slower here -- the ACT
            # table load occupies it first; gpsimd-queue load also regressed)
            c0 = NIDX * CHUNK_WIDTHS[0]
            nc.sync.dma_start(out=idx_t[:, 0:c0], in_=idx_d.ap()[:, 0:c0])
            nc.sync.dma_start(out=idx_t[:, c0:NIDX * TILES],
                              in_=idx_d.ap()[:, c0:NIDX * TILES])

            # one gather tile for the whole batch: 256 rows per partition
            g_t = gpool.tile([P, NIDX * TILES * DIM], f32)

            # running per-partition softplus accumulator (ping-pong tiles so
            # each chunk's accumulate is out-of-place)
            racc0 = accp.tile([P, 1], f32, tag="racc0")
            racc1 = accp.tile([P, 1], f32, tag="racc1")
            racc = [racc0, racc1]
            ones = accp.tile([P, 1], f32)
            nc.vector.memset(ones[:], 1.0)

            def gather(col):
                nc.gpsimd.indirect_dma_start(
                    out=g_t[:, col * DIM:(col + 1) * DIM],
                    out_offset=None,
                    in_=uw_w.ap(),
                    in_offset=bass.IndirectOffsetOnAxis(
                        ap=idx_t[:, col:col + 1], axis=0),
                )

            base = 0  # gather-block / idx column offset
            nacc = 0  # chunks accumulated so far
            for ci, W in enumerate(CHUNK_WIDTHS[:-1]):
                blks = NIDX * W
                # one gather per (c, t) block; host orders idx columns
                # chunk-major, then c-major, t-minor; u rows then pos_w/neg_w
                for j in range(blks):
                    gather(base + j)
                u4 = g_t[:, base * DIM:(base + CTX * W) * DIM].rearrange(
                    "p (c t d) -> p c t d", c=CTX, t=W)
                w4 = g_t[:, (base + CTX * W) * DIM:(base + blks) * DIM].rearrange(
                    "p (c t d) -> p c t d", c=WK, t=W)

                # context sum over c=10: tree 10 -> 5 -> (4->2->1) + leftover
                s1 = work.tile([P, 5 * W * DIM], f32, tag="s1")
                s1v = s1[:].rearrange("p (c t d) -> p c t d", c=5, t=W)
                nc.vector.tensor_tensor(out=s1v[:, :, :, :], in0=u4[:, 0:5], in1=u4[:, 5:10], op=ADD)
                s2 = work.tile([P, 2 * W * DIM], f32, tag="s2")
                s2v = s2[:].rearrange("p (c t d) -> p c t d", c=2, t=W)
                nc.vector.tensor_tensor(out=s2v[:, :, :, :], in0=s1v[:, 0:2], in1=s1v[:, 2:4], op=ADD)
                s3 = work.tile([P, W * DIM], f32, tag="s3")
                s3v = s3[:].rearrange("p (o t d) -> p o t d", o=1, t=W)
                nc.vector.tensor_tensor(out=s3v[:, :, :, :], in0=s2v[:, 0:1], in1=s2v[:, 1:2], op=ADD)
                su = work.tile([P, W * DIM], f32, tag="su")
                suv = su[:].rearrange("p (o t d) -> p o t d", o=1, t=W)
                nc.vector.tensor_tensor(out=suv[:, :, :, :], in0=s3v[:, :, :, :], in1=s1v[:, 4:5], op=ADD)

                # negative-sample sum over c=1..5: 4 -> 2 -> 1, + leftover
                n1 = work.tile([P, 2 * W * DIM], f32, tag="n1")
                n1v = n1[:].rearrange("p (c t d) -> p c t d", c=2, t=W)
                nc.vector.tensor_tensor(out=n1v[:, :, :, :], in0=w4[:, 1:3], in1=w4[:, 3:5], op=ADD)
                n2 = work.tile([P, W * DIM], f32, tag="n2")
                n2v = n2[:].rearrange("p (o t d) -> p o t d", o=1, t=W)
                nc.vector.tensor_tensor(out=n2v[:, :, :, :], in0=n1v[:, 0:1], in1=n1v[:, 1:2], op=ADD)
                wneg = work.tile([P, W * DIM], f32, tag="wneg")
                wnv = wneg[:].rearrange("p (o t d) -> p o t d", o=1, t=W)
                nc.vector.tensor_tensor(out=wnv[:, :, :, :], in0=n2v[:, :, :, :], in1=w4[:, 5:6], op=ADD)

                # per-sample dot products
                prod = work.tile([P, 2 * W * DIM], f32, tag="prod")
                pv = prod[:].rearrange("p (k t d) -> p k t d", k=2, t=W)
                nc.vector.tensor_tensor(out=pv[:, 0:1], in0=suv[:, :, :, :], in1=w4[:, 0:1], op=MUL)
                nc.vector.tensor_tensor(out=pv[:, 1:2], in0=suv[:, :, :, :], in1=wnv[:, :, :, :], op=MUL)
                sc = work.tile([P, 2 * W], f32, tag="sc")
                sv = sc[:].rearrange("p (k t) -> p k t", k=2)
                nc.vector.tensor_reduce(
                    out=sv[:, 0:1, :], in_=pv[:, 0:1],
                    axis=mybir.AxisListType.X, op=ADD, negate=True)
                nc.vector.tensor_reduce(
                    out=sv[:, 1:2, :], in_=pv[:, 1:2],
                    axis=mybir.AxisListType.X, op=ADD)

                # per-chunk softplus + row accumulate (hidden under the gather
                # stream; after the final gather only the last chunk's chain
                # remains). softplus(x) = relu(x) + log1p(exp(-|x|)); the
                # native Softplus enum has no activation table on this arch.
                relu = work.tile([P, 2 * W], f32, tag="relu")
                nc.vector.tensor_scalar_max(relu[:], sc[:], 0.0)
                nabs = work.tile([P, 2 * W], f32, tag="nabs")
                nc.vector.scalar_tensor_tensor(  # -|x| = sc - 2*relu
                    out=nabs[:], in0=relu[:], scalar=-2.0, in1=sc[:],
                    op0=MUL, op1=ADD)
                ex = work.tile([P, 2 * W], f32, tag="ex")
                nc.scalar.activation(ex[:], nabs[:],
                                     mybir.ActivationFunctionType.Exp)
                ln = work.tile([P, 2 * W], f32, tag="ln")
                nc.scalar.activation(ln[:], ex[:],
                                     mybir.ActivationFunctionType.Ln, bias=1.0)
                sp = work.tile([P, 2 * W], f32, tag="sp")
                nc.vector.tensor_tensor(out=sp[:], in0=relu[:], in1=ln[:], op=ADD)
                if nacc == 0:
                    nc.vector.tensor_reduce(
                        out=racc[0][:], in_=sp[:],
                        axis=mybir.AxisListType.X, op=ADD)
                else:
                    rp = work.tile([P, 1], f32, tag="rp")
                    nc.vector.tensor_reduce(out=rp[:], in_=sp[:],
                                            axis=mybir.AxisListType.X, op=ADD)
                    nc.vector.tensor_tensor(
                        out=racc[nacc % 2][:], in0=racc[(nacc - 1) % 2][:],
                        in1=rp[:], op=ADD)
                nacc += 1
                base += blks

            # --- final chunk (W == 1), host column order [u0..u9, n0..n4,
            # pos]: emission is interleaved with the gathers so the context
            # and neg-sample sums (and the whole neg-score softplus chain)
            # run while the remaining rows gather; the pos-w row is gathered
            # LAST, leaving only prod->reduce->softplus(1 col)->acc->PE->out
            # after the final gather.
            assert CHUNK_WIDTHS[-1] == 1
            for j in range(CTX):
                gather(base + j)
            uf = g_t[:, base * DIM:(base + CTX) * DIM].rearrange(
                "p (c d) -> p c d", c=CTX)
            f1 = work.tile([P, 5 * DIM], f32, tag="s1")
            f1v = f1[:].rearrange("p (c d) -> p c d", c=5)
            nc.vector.tensor_tensor(out=f1v[:, :, :], in0=uf[:, 0:5], in1=uf[:, 5:10], op=ADD)
            f2 = work.tile([P, 2 * DIM], f32, tag="s2")
            f2v = f2[:].rearrange("p (c d) -> p c d", c=2)
            nc.vector.tensor_tensor(out=f2v[:, :, :], in0=f1v[:, 0:2], in1=f1v[:, 2:4], op=ADD)
            f3 = work.tile([P, DIM], f32, tag="s3")
            f3v = f3[:].rearrange("p (o d) -> p o d", o=1)
            nc.vector.tensor_tensor(out=f3v[:, :, :], in0=f2v[:, 0:1], in1=f2v[:, 1:2], op=ADD)
            fsu = work.tile([P, DIM], f32, tag="su")
            fsuv = fsu[:].rearrange("p (o d) -> p o d", o=1)
            nc.vector.tensor_tensor(out=fsuv[:, :, :], in0=f3v[:, :, :], in1=f1v[:, 4:5], op=ADD)

            for j in range(NEG):
                gather(base + CTX + j)
            wf = g_t[:, (base + CTX) * DIM:(base + CTX + NEG) * DIM].rearrange(
                "p (c d) -> p c d", c=NEG)
            fn1 = work.tile([P, 2 * DIM], f32, tag="n1")
            fn1v = fn1[:].rearrange("p (c d) -> p c d", c=2)
            nc.vector.tensor_tensor(out=fn1v[:, :, :], in0=wf[:, 0:2], in1=wf[:, 2:4], op=ADD)
            fn2 = work.tile([P, DIM], f32, tag="n2")
            fn2v = fn2[:].rearrange("p (o d) -> p o d", o=1)
            nc.vector.tensor_tensor(out=fn2v[:, :, :], in0=fn1v[:, 0:1], in1=fn1v[:, 1:2], op=ADD)
            fwn = work.tile([P, DIM], f32, tag="wneg")
            fwnv = fwn[:].rearrange("p (o d) -> p o d", o=1)
            nc.vector.tensor_tensor(out=fwnv[:, :, :], in0=fn2v[:, :, :], in1=wf[:, 4:5], op=ADD)

            # neg-score chain (emitted before the pos gather so its sem wait
            # cannot be group-coalesced with the pos gather's completion)
            pn = work.tile([P, DIM], f32, tag="prod")
            nc.vector.tensor_tensor(out=pn[:], in0=fsu[:], in1=fwn[:], op=MUL)
            scn = work.tile([P, 1], f32, tag="scn")
            nc.vector.tensor_reduce(out=scn[:], in_=pn[:],
                                    axis=mybir.AxisListType.X, op=ADD)
            reln = work.tile([P, 1], f32, tag="reln")
            nc.vector.tensor_scalar_max(reln[:], scn[:], 0.0)
            nabn = work.tile([P, 1], f32, tag="nabn")
            nc.vector.scalar_tensor_tensor(out=nabn[:], in0=reln[:],
                                           scalar=-2.0, in1=scn[:],
                                           op0=MUL, op1=ADD)
            exn = work.tile([P, 1], f32, tag="exn")
            nc.scalar.activation(exn[:], nabn[:],
                                 mybir.ActivationFunctionType.Exp)
            lnn = work.tile([P, 1], f32, tag="lnn")
            nc.scalar.activation(lnn[:], exn[:],
                                 mybir.ActivationFunctionType.Ln, bias=1.0)
            spn = work.tile([P, 1], f32, tag="spn")
            nc.vector.tensor_tensor(out=spn[:], in0=reln[:], in1=lnn[:], op=ADD)

            # cross-partition sum via PSUM-accumulating matmuls: the generic
            # chunks' accumulator and the neg column fold in early (PE is
            # idle); only the pos column's matmul trails the final gather.
            ps = psp.tile([1, 1], f32)
            nc.tensor.matmul(ps[:], lhsT=racc[(nacc - 1) % 2][:], rhs=ones[:],
                             start=True, stop=False)
            nc.tensor.matmul(ps[:], lhsT=spn[:], rhs=ones[:],
                             start=False, stop=False)

            pos_col = base + CTX + NEG
            gather(pos_col)

            # pos-score chain -- the only work after the final gather
            pp = work.tile([P, DIM], f32, tag="prodp")
            nc.vector.tensor_tensor(
                out=pp[:], in0=fsu[:],
                in1=g_t[:, pos_col * DIM:(pos_col + 1) * DIM], op=MUL)
            scp = work.tile([P, 1], f32, tag="scp")
            nc.vector.tensor_reduce(out=scp[:], in_=pp[:],
                                    axis=mybir.AxisListType.X, op=ADD,
                                    negate=True)
            relp = work.tile([P, 1], f32, tag="relp")
            nc.vector.tensor_scalar_max(relp[:], scp[:], 0.0)
            nabp = work.tile([P, 1], f32, tag="nabp")
            nc.vector.scalar_tensor_tensor(out=nabp[:], in0=relp[:],
                                           scalar=-2.0, in1=scp[:],
                                           op0=MUL, op1=ADD)
            exp_ = work.tile([P, 1], f32, tag="exp")
            nc.scalar.activation(exp_[:], nabp[:],
                                 mybir.ActivationFunctionType.Exp)
            lnp = work.tile([P, 1], f32, tag="lnp")
            nc.scalar.activation(lnp[:], exp_[:],
                                 mybir.ActivationFunctionType.Ln, bias=1.0)
            spp = work.tile([P, 1], f32, tag="spp")
            nc.vector.tensor_tensor(out=spp[:], in0=relp[:], in1=lnp[:], op=ADD)
            nc.tensor.matmul(ps[:], lhsT=spp[:], rhs=ones[:],
                             start=False, stop=True)
            res_sb = accp.tile([1, 1], f32)
            nc.vector.tensor_copy(out=res_sb[:], in_=ps[:])
            nc.sync.dma_start(out=out_d.ap(), in_=res_sb[:])

    # Exp and Ln both live in the natural_log_exp_and_others table set, but
    # the greedy table chooser picks exp_and_others for Exp and natural_log
    # for Ln, putting a ~2.7us table swap in the kernel's serial tail. Empty
    # those two sets (positions preserved -- act_func_set_id is positional)
    # during compile so both funcs resolve to the combined table.
    orig_tables = bacc.get_activation_tables

    def _tables_combined(arch):
        t = dict(orig_tables(arch))
        if "natural_log_exp_and_others" in t:
            for k in ("exp_and_others", "natural_log"):
                if k in t:
                    t[k] = frozenset()
        return t

    bacc.get_activation_tables = _tables_combined
    try:
        nc.compile()
    finally:
        bacc.get_activation_tables = orig_tables
    return nc


def _get_nc():
    if "nc" not in _CACHE:
        _CACHE["nc"] = _build_nc()
    return _CACHE["nc"]


def _make_in_maps(pos_u, pos_w, neg_w, u_weight, w_weight):
    pos_u = np.asarray(pos_u)
    pos_w = np.asarray(pos_w)
    neg_w = np.asarray(neg_w)
    uw = np.ascontiguousarray(
        np.concatenate([np.asarray(u_weight, dtype=np.float32),
                        np.asarray(w_weight, dtype=np.float32)], axis=0))

    in_maps = []
    for c in range(N_CORES):
        sl = slice(c * BPC, (c + 1) * BPC)
        # per-sample 16 indices: u c=0..9 then w k=0..5 (+VOCAB offset into
        # the concatenated table)
        all_ind = np.concatenate(
            [np.asarray(pos_u[sl], dtype=np.int32),
             np.asarray(pos_w[sl], dtype=np.int32)[:, None] + VOCAB,
             np.asarray(neg_w[sl], dtype=np.int32) + VOCAB], axis=1)  # [2048, 16]
        # device layout: columns chunk-major, within a chunk of width W the
        # column for lookup j of sample s = (t0 + t)*128 + p is j*W + t
        A = all_ind.reshape(TILES, P, NIDX)  # [t_global, p, j]
        # final chunk: [u0..u9, neg0..neg4, pos] so the pos-w row is the
        # very last gather (device gathers columns in order)
        final_order = list(range(CTX)) + list(range(CTX + 1, NIDX)) + [CTX]
        cols = []
        t0 = 0
        for ci, W in enumerate(CHUNK_WIDTHS):
            blk = A[t0:t0 + W]                      # [W, p, j]
            if ci == len(CHUNK_WIDTHS) - 1:
                blk = blk[:, :, final_order]
            cols.append(blk.transpose(1, 2, 0).reshape(P, NIDX * W))
            t0 += W
        idx = np.concatenate(cols, axis=1)  # [P, NIDX*TILES]
        in_maps.append({
            "idx": np.ascontiguousarray(idx),
            "uw_weight": uw,
        })
    return in_maps


def kernel(pos_u, pos_w, neg_w, u_weight, w_weight):
    from concourse.bass_utils import run_bass_kernel_spmd

    nc = _get_nc()
    in_maps = _make_in_maps(pos_u, pos_w, neg_w, u_weight, w_weight)
    res = run_bass_kernel_spmd(nc, in_maps, core_ids=list(range(N_CORES)))
    total = sum(float(r["out"][0, 0]) for r in res.results)
    return np.asarray(total, dtype=np.float32)

